# revision 31
# baseline (speedup 1.0000x reference)
"""Trainium2 Bass kernel for MiniBatch Edge-Conditioned Conv (2 blocks + classifier).

Reference computation:
  block(h, ef, We, be, Wn, bn, src, dst, nid, n_dst):
    e   = relu(ef @ We + be).reshape(E, H, D)      # per-edge weights
    m   = einsum('ehd,ed->eh', e, h[src])          # per-edge matvec
    agg = segment_sum(m, dst, n_dst)
    return agg + relu(h[nid] @ Wn + bn)
  out = block1(block0(nf)) @ Wfc + bfc

Sharding: edges sorted by dst, sharded by dst-range across 8 cores (so the
segment-sum is core-local).  h1 is AllGathered between blocks.

Device pipeline per 128-edge chunk (d-on-partition layout):
  PE:   G[(h2,d), e] = WeR_slice.T @ eftT   (32 matmuls, [17,128]@[17,128])
  V:    T = relu(G) * hsT2  (hsT2 = gathered h[src] transposed+duplicated,
        built entirely on the DMA engines; relu+mult split across
        ACT(+DVE mult) / DVE-STT / Pool-STT by a weighted schedule)
  PE:   m[e, 2t:2t+2] = T_tile.T @ S    (data-stationary d-reduce, 2 rows out)
  ACT:  m_sb = copy(m)  (PSUM f32 -> SBUF bf16)
  PE:   seg[v, h] += onehot.T @ m_sb    (64-row segment-sum, accumulated
        across the tile's chunks in PSUM)
Node update / classifier as small per-tile epilogues.
"""

import sys

sys.path.insert(0, "/opt/trn_rl_repo")

import numpy as np
import ml_dtypes

import concourse.bass as bass
import concourse.mybir as mybir
import concourse.tile as tile
from concourse import bacc, bass_utils

BF16 = ml_dtypes.bfloat16

# Problem constants (hardcoded per harness contract)
N0, N1, N2 = 102400, 10240, 1024
D_IN, E_IN, H, C = 64, 16, 64, 10
E0, E1 = 102400, 10240
NCORES = 8
P = 128
HD = H * D_IN  # 4096
NSUB = 8       # evac units per chunk ([128, 512] PSUM gen tiles)
TPS = 4        # h-pair tiles per unit

# prefetch lookahead for the hs gather->transpose DMA chain
GB = 4         # chunks per batched gather (indices pre-duplicated host-side)
LA_GB = 4      # gather batches of lookahead
LA_TRANS = 6   # chunks of transpose lookahead
PEND_MIN = 9   # keep >=2 chunks of B-step backlog so PE never waits on evac

PAD_SENTINEL = 200.0
DIAG_NO_CC = False

# evac engine schedule weights (act, pool, dve) — tuned against TimelineSim
SCHED_W = (22, 16, 12)


def _sched_stream():
    """Error-diffusion weighted round-robin over ('act', 'pool', 'dve')."""
    names = ("act", "pool", "dve")
    w = np.array(SCHED_W, dtype=np.float64)
    w = w / w.sum()
    credit = np.zeros(3)
    while True:
        credit += w
        i = int(np.argmax(credit))
        credit[i] -= 1.0
        yield names[i]


def _prep_edges(ef, src, dst, n_dst_per_core, tiles_per_core):
    """Sort edges by dst, shard by dst-range, pad per (core,tile) to chunks of 128.

    Returns per-core arrays + per-tile chunk counts (shared by cores).
    """
    E = ef.shape[0]
    core = dst // n_dst_per_core
    tloc = (dst % n_dst_per_core) // P
    dloc = dst % P

    counts = np.zeros((NCORES, tiles_per_core), dtype=np.int64)
    np.add.at(counts, (core, tloc), 1)
    cpts = np.maximum(1, np.ceil(counts.max(axis=0) / P).astype(np.int64))  # [T]
    offs = np.concatenate([[0], np.cumsum(cpts)])  # chunk offsets per tile
    total_chunks = int(offs[-1])
    EP = total_chunks * P

    order = np.lexsort((dloc, tloc, core))
    sc, st = core[order], tloc[order]
    eftA = np.zeros((NCORES, 17, EP), dtype=BF16)
    srcA = np.zeros((NCORES, P, total_chunks), dtype=np.int32)
    dstA = np.full((NCORES, P, total_chunks), PAD_SENTINEL, dtype=np.float32)

    ef16 = ef.astype(BF16)
    idx_all = np.arange(E)
    for c in range(NCORES):
        for t in range(tiles_per_core):
            sel = order[(sc == c) & (st == t)]
            n = len(sel)
            col0 = int(offs[t]) * P
            # column layout: edge j -> (p=j%P, chunk=j//P); eft col = ch*P + p
            eftA[c, :16, col0 : col0 + n] = ef16[sel].T
            eftA[c, 16, col0 : col0 + n] = 1.0
            ch = idx_all[:n] // P
            pp = idx_all[:n] % P
            srcA[c, pp, int(offs[t]) + ch] = src[sel]
            dstA[c, pp, int(offs[t]) + ch] = (dst[sel] % P).astype(np.float32)
    return eftA, srcA, dstA, cpts, offs, EP, total_chunks


def _augment(W, b):
    return np.concatenate([W, b[None, :]], axis=0).astype(BF16)


def _build_program(cpts0, offs0, EP0, TC0, cpts1, offs1, EP1, TC1):
    """Build the SPMD Bass program (same NEFF for all 8 cores)."""
    nc = bacc.Bacc(
        "TRN2", target_bir_lowering=False, debug=False,
        num_devices=1 if DIAG_NO_CC else NCORES,
    )
    dt = mybir.dt
    T0 = N1 // NCORES // P  # 10 dst tiles per core, block 0

    # ---- I/O ----
    i_we0 = nc.dram_tensor("we0a", [17, HD], dt.bfloat16, kind="ExternalInput")
    i_we1 = nc.dram_tensor("we1a", [17, HD], dt.bfloat16, kind="ExternalInput")
    i_wn0 = nc.dram_tensor("wn0a", [D_IN + 1, H], dt.bfloat16, kind="ExternalInput")
    i_wn1 = nc.dram_tensor("wn1a", [H + 1, H], dt.bfloat16, kind="ExternalInput")
    i_wfc = nc.dram_tensor("wfca", [H + 1, C], dt.bfloat16, kind="ExternalInput")
    i_nf = nc.dram_tensor("nf16", [N0, D_IN], dt.bfloat16, kind="ExternalInput")
    i_eft0 = nc.dram_tensor("eft0", [17, EP0], dt.bfloat16, kind="ExternalInput")
    i_src0 = nc.dram_tensor("src0i", [P, 2 * TC0], dt.int32, kind="ExternalInput")
    i_oh0 = nc.dram_tensor("oh0", [P, EP0], dt.bfloat16, kind="ExternalInput")
    i_eft1 = nc.dram_tensor("eft1", [17, EP1], dt.bfloat16, kind="ExternalInput")
    i_src1 = nc.dram_tensor("src1i", [P, 2 * TC1], dt.int32, kind="ExternalInput")
    i_oh1 = nc.dram_tensor("oh1", [P, EP1], dt.bfloat16, kind="ExternalInput")
    i_nid0 = nc.dram_tensor("nidx0", [P, 2 * T0], dt.int32, kind="ExternalInput")
    i_nid1 = nc.dram_tensor("nidx1", [P, 2], dt.int32, kind="ExternalInput")
    i_ident = nc.dram_tensor("ident", [P, P], dt.bfloat16, kind="ExternalInput")
    i_ssel = nc.dram_tensor("ssel", [P, 2], dt.bfloat16, kind="ExternalInput")
    o_out = nc.dram_tensor("out", [P, C], dt.float32, kind="ExternalOutput")

    RELU = mybir.ActivationFunctionType.Relu
    COPY = mybir.ActivationFunctionType.Copy
    MULT = mybir.AluOpType.mult
    MAX = mybir.AluOpType.max
    ADD = mybir.AluOpType.add

    sched = _sched_stream()

    with tile.TileContext(nc) as tc:
        with (
            tc.tile_pool(name="const", bufs=1) as cpool,
            tc.tile_pool(name="dram", bufs=1, space="DRAM") as dpool,
        ):
            we0_s = cpool.tile([17, HD], dt.bfloat16)
            nc.sync.dma_start(we0_s[:], i_we0[:])
            we1_s = cpool.tile([17, HD], dt.bfloat16)
            nc.sync.dma_start(we1_s[:], i_we1[:])
            wn0_s = cpool.tile([D_IN + 1, H], dt.bfloat16)
            nc.sync.dma_start(wn0_s[:], i_wn0[:])
            wn1_s = cpool.tile([H + 1, H], dt.bfloat16)
            nc.sync.dma_start(wn1_s[:], i_wn1[:])
            wfc_s = cpool.tile([H + 1, C], dt.bfloat16)
            nc.sync.dma_start(wfc_s[:], i_wfc[:])
            ident_s = cpool.tile([P, P], dt.bfloat16)
            nc.sync.dma_start(ident_s[:], i_ident[:])
            ssel_s = cpool.tile([P, 2], dt.bfloat16)
            nc.sync.dma_start(ssel_s[:], i_ssel[:])

            h1s = dpool.tile([N1 // NCORES, H], dt.bfloat16)  # own slice
            h1f = dpool.tile([N1, H], dt.bfloat16)  # all-gathered

            def edge_phase(Ttiles, cpts, offs, eft_in, src_in, oh_in, we_s,
                           gather_dram, nid_in, tile_out_cb):
                """Edge pipeline.  All DMA work (hs gather -> dup -> transpose,
                tile inputs, node-update operand prep) is prefetched several
                chunks ahead so the in-order compute queues never wait on DMA
                latency.  The PE d-reduce/segment stage of chunk c is emitted
                interleaved into chunk c+1's phase A (software pipelining).
                Per dst tile t, tile_out_cb(t, seg_psum, npool, spsum, nfT2)
                runs once the tile's segment sum is complete."""
                with (
                    tc.tile_pool(name="chunkin", bufs=2) as chpool,
                    tc.tile_pool(name="hsp", bufs=10) as hspool,
                    tc.tile_pool(name="hstp", bufs=10) as hstpool,
                    tc.tile_pool(name="tsb", bufs=24) as tpool,
                    tc.tile_pool(name="rsb", bufs=4) as rpool,
                    tc.tile_pool(name="msb", bufs=3) as mpool,
                    tc.tile_pool(name="nu", bufs=2) as npool,
                    tc.tile_pool(name="genps", bufs=6, space="PSUM") as gpool,
                    tc.tile_pool(name="smallps", bufs=1, space="PSUM") as spsum,
                    tc.tile_pool(name="segps", bufs=1, space="PSUM") as segpool,
                ):
                    nidd_c = npool.tile([P, 2 * Ttiles], dt.int32, tag="nid")
                    nc.sync.dma_start(nidd_c[:], nid_in[:])

                    flat = [(t, ch) for t in range(Ttiles)
                            for ch in range(int(cpts[t]))]
                    nflat = len(flat)
                    # gather batches: (t, ch_start, n_chunks), within tiles
                    gblist = []
                    for t in range(Ttiles):
                        cpt = int(cpts[t])
                        for b0 in range(0, cpt, GB):
                            gblist.append((t, b0, min(GB, cpt - b0)))
                    tin = {}     # t -> [eft_c, srcd_c, oh_c]
                    nfpre = {}   # t -> nfT2 (node-update operand, transposed)
                    hsb_map = {}   # (t, batch) -> gathered [P, n*128] tile
                    hsT2_map = {}  # flat idx -> [P, P] transposed operand
                    tsteps = {}  # t -> completed prefetch steps
                    nfg_map = {}

                    def ensure_step(t):
                        # staged: emit one prefetch DMA per call so the SP
                        # sequencer (565ns per DMA config) never bursts
                        if t >= Ttiles:
                            return
                        step = tsteps.get(t, 0)
                        if step >= 5:
                            return
                        tsteps[t] = step + 1
                        cpt = int(cpts[t])
                        c0 = int(offs[t])
                        if step == 0:
                            eft_c = chpool.tile([17, cpt * P], dt.bfloat16,
                                                tag="eft", name="eft_c")
                            nc.sync.dma_start(
                                eft_c[:], eft_in[:, c0 * P : (c0 + cpt) * P]
                            )
                            srcd_c = chpool.tile([P, 2 * cpt], dt.int32,
                                                 tag="src", name="srcd_c")
                            nc.sync.dma_start(
                                srcd_c[:], src_in[:, 2 * c0 : 2 * (c0 + cpt)]
                            )
                            tin[t] = [eft_c, srcd_c, None]
                        elif step == 1:
                            oh_c = chpool.tile([P, cpt * P], dt.bfloat16,
                                               tag="oh", name="oh_c")
                            nc.sync.dma_start(
                                oh_c[:], oh_in[:, c0 * P : (c0 + cpt) * P]
                            )
                            tin[t][2] = oh_c
                        elif step == 2:
                            # node-update operand: gather h[nid] twice
                            # (duplicated indices)
                            nfg2 = npool.tile([P, P], dt.bfloat16, tag="nfg2",
                                              bufs=3)
                            nc.gpsimd.indirect_dma_start(
                                out=nfg2[:].rearrange("p (j d) -> p j d", j=2),
                                out_offset=None,
                                in_=gather_dram[:],
                                in_offset=bass.IndirectOffsetOnAxis(
                                    ap=nidd_c[:, 2 * t : 2 * t + 2], axis=0
                                ),
                            )
                            nfg_map[t] = nfg2
                        elif step == 3:
                            nfT2 = npool.tile([P, P], dt.bfloat16, tag="nfT2",
                                              bufs=3)
                            nc.sync.dma_start_transpose(
                                nfT2[:], nfg_map.pop(t)[:]
                            )
                            nfpre[t] = nfT2

                    def ensure_inputs(t):
                        # force-complete all prefetch steps for tile t
                        while t < Ttiles and tsteps.get(t, 0) < 5:
                            ensure_step(t)

                    def issue_gbatch(bi):
                        if bi >= len(gblist):
                            return
                        t, b0, nch = gblist[bi]
                        ensure_inputs(t)
                        srcd_c = tin[t][1]
                        hsb = hspool.tile([P, nch * P], dt.bfloat16, tag="hsb",
                                          name="hsb")
                        nc.gpsimd.indirect_dma_start(
                            out=hsb[:].rearrange("p (j d) -> p j d", j=2 * nch),
                            out_offset=None,
                            in_=gather_dram[:],
                            in_offset=bass.IndirectOffsetOnAxis(
                                ap=srcd_c[:, 2 * b0 : 2 * (b0 + nch)], axis=0
                            ),
                        )
                        hsb_map[(t, b0 // GB)] = hsb

                    def issue_trans(i):
                        if i >= nflat:
                            return
                        t, ch = flat[i]
                        hsb = hsb_map[(t, ch // GB)]
                        off = (ch % GB) * P
                        hsT2 = hstpool.tile([P, P], dt.bfloat16, tag="hsT2")
                        nc.sync.dma_start_transpose(
                            hsT2[:], hsb[:, off : off + P]
                        )
                        hsT2_map[i] = hsT2

                    # prime the prefetch pipeline
                    for j in range(1 + LA_GB):
                        issue_gbatch(j)
                    for j in range(LA_TRANS):
                        issue_trans(j)
                    next_gb = 1 + LA_GB

                    pend = []  # closures for prior chunks' B-steps

                    def run_pend(k, lag=True):
                        while pend and k > 0 and (
                            not lag or len(pend) > PEND_MIN
                        ):
                            pend.pop(0)()
                            k -= 1

                    seg_map = {}
                    for i, (t, ch) in enumerate(flat):
                        cpt = int(cpts[t])
                        if ch == 0:
                            seg_map[t] = segpool.tile([P, H], dt.float32,
                                                      tag="seg", name="seg")
                            pass
                        seg = seg_map[t]
                        if ch % GB == 0:
                            issue_gbatch(next_gb)
                            next_gb += 1
                        issue_trans(i + LA_TRANS)
                        ensure_step(t + 1)  # staggered next-tile prefetch
                        run_pend(1)
                        eft_c, src_c, oh_c = tin[t]
                        hsT2 = hsT2_map.pop(i)
                        m = spsum.tile([P, H], dt.float32, tag="m")
                        tsb_list = []
                        for s in range(NSUB):
                            g = gpool.tile([P, TPS * P], dt.float32, tag="g")
                            for u in range(TPS):
                                tt = s * TPS + u
                                nc.tensor.matmul(
                                    g[:, u * P : (u + 1) * P],
                                    lhsT=we_s[:, tt * P : (tt + 1) * P],
                                    rhs=eft_c[:, ch * P : (ch + 1) * P],
                                    start=True,
                                    stop=True,
                                )
                            Tsb = tpool.tile([P, TPS * P], dt.bfloat16, tag="T")
                            g3 = g[:].rearrange("p (o e) -> p o e", o=TPS)
                            T3 = Tsb[:].rearrange("p (o e) -> p o e", o=TPS)
                            h3 = (
                                hsT2[:]
                                .rearrange("p (o e) -> p o e", o=1)
                                .to_broadcast([P, TPS, P])
                            )
                            eng = next(sched)
                            if eng == "act":
                                R = rpool.tile([P, TPS * P], dt.bfloat16,
                                               tag="R")
                                nc.scalar.activation(R[:], g[:], RELU)
                                nc.vector.tensor_tensor(
                                    out=T3,
                                    in0=R[:].rearrange("p (o e) -> p o e",
                                                       o=TPS),
                                    in1=h3,
                                    op=MULT,
                                )
                            elif eng == "dve":
                                nc.vector.scalar_tensor_tensor(
                                    out=T3, in0=g3, scalar=0.0, in1=h3,
                                    op0=MAX, op1=MULT,
                                )
                            else:  # pool
                                nc.gpsimd.scalar_tensor_tensor(
                                    out=T3, in0=g3, scalar=0.0, in1=h3,
                                    op0=MAX, op1=MULT,
                                )
                            tsb_list.append(Tsb)
                            if s % 2 == 1:
                                run_pend(1)

                        # --- queue phase B for this chunk ---
                        def make_b(tsb_list=tsb_list, m=m, oh_c=oh_c, ch=ch,
                                   seg=seg, first=(ch == 0),
                                   last=(ch == cpt - 1)):
                            st = {}

                            def b_dred(lo, hi):
                                def f():
                                    for s in range(lo, hi):
                                        Tsb = tsb_list[s]
                                        for u in range(TPS):
                                            tt = s * TPS + u
                                            nc.tensor.matmul(
                                                m[:, 2 * tt : 2 * tt + 2],
                                                lhsT=Tsb[:, u * P : (u + 1) * P],
                                                rhs=ssel_s[:],
                                                start=True,
                                                stop=True,
                                            )
                                return f

                            def b_mevac():
                                msb = mpool.tile([P, H], dt.bfloat16,
                                                 tag="msb")
                                nc.scalar.activation(msb[:], m[:], COPY)
                                st["msb"] = msb

                            def b_seg():
                                nc.tensor.matmul(
                                    seg[:],
                                    lhsT=oh_c[:, ch * P : (ch + 1) * P],
                                    rhs=st["msb"][:],
                                    start=first,
                                    stop=last,
                                )
                            return [b_dred(0, NSUB // 2),
                                    b_dred(NSUB // 2, NSUB),
                                    b_mevac, b_seg]

                        pend.extend(make_b())

                        if ch == cpt - 1:
                            # queue tile epilogue; runs interleaved with the
                            # next tile's phase A (after this tile's B-steps)
                            def make_ep(t=t, seg=seg):
                                def ep():
                                    # evacuate seg psum first so the next
                                    # tile's segment accumulation can start
                                    agg = npool.tile([P, H], dt.float32,
                                                     tag="agg", bufs=2,
                                                     name="agg")
                                    nc.vector.tensor_copy(agg[:], seg[:])
                                    tile_out_cb(t, agg, npool, gpool,
                                                nfpre.pop(t))
                                return ep

                            pend.append(make_ep())

                    # end of phase: drain everything
                    run_pend(len(pend), lag=False)

            def node_update(t, seg, npool, gpool, nfT2, wn_s, out_cb):
                """h_out = seg + relu(gather[nid] @ Wn_aug); out_cb(t, hout)."""
                nfgT = npool.tile([D_IN + 1, P], dt.bfloat16, tag="nfgT")
                nc.vector.tensor_copy(nfgT[:D_IN, :], nfT2[:D_IN, :])
                nc.vector.memset(nfgT[D_IN : D_IN + 1, :], 1.0)
                nup = gpool.tile([P, H], dt.float32, tag="g", name="nup")
                nc.tensor.matmul(
                    nup[:], lhsT=nfgT[:], rhs=wn_s[:], start=True, stop=True
                )
                nur = npool.tile([P, H], dt.float32, tag="nur")
                nc.scalar.activation(nur[:], nup[:], RELU)
                hout = npool.tile([P, H], dt.float32, tag="hout")
                nc.vector.tensor_tensor(
                    out=hout[:], in0=nur[:], in1=seg[:], op=ADD
                )
                out_cb(t, hout, npool, gpool)

            # ================= BLOCK 0 =================
            def b0_tile(t, seg, npool, gpool, nfT2):
                def b0_out(t, hout, npool, gpool):
                    hb = npool.tile([P, H], dt.bfloat16, tag="hb")
                    nc.vector.tensor_copy(hb[:], hout[:])
                    nc.sync.dma_start(h1s[t * P : (t + 1) * P, :], hb[:])

                node_update(t, seg, npool, gpool, nfT2, wn0_s, b0_out)

            edge_phase(T0, cpts0, offs0, i_eft0, i_src0, i_oh0, we0_s,
                       i_nf, i_nid0, b0_tile)

            # ================= ALLGATHER h1 =================
            if DIAG_NO_CC:
                nc.sync.dma_start(h1f[0 : N1 // NCORES, :], h1s[:])
                nc.sync.dma_start(h1f[N1 // NCORES :, :],
                                  h1f[0 : N1 - N1 // NCORES, :])
            else:
                nc.gpsimd.collective_compute(
                    "AllGather",
                    mybir.AluOpType.bypass,
                    replica_groups=[list(range(NCORES))],
                    ins=[h1s[:].opt()],
                    outs=[h1f[:].opt()],
                )

            # ================= BLOCK 1 =================
            def b1_tile(t, seg, npool, gpool, nfT2):
                def b1_out(t, hout, npool, gpool):
                    hb = npool.tile([P, H], dt.bfloat16, tag="hb2")
                    nc.vector.tensor_copy(hb[:], hout[:])
                    trp2 = gpool.tile([H, P], dt.bfloat16, tag="g", name="trp2")
                    nc.tensor.transpose(trp2[:], hb[:], ident_s[:])
                    h2T = npool.tile([H + 1, P], dt.bfloat16, tag="h2T")
                    nc.vector.tensor_copy(h2T[:H, :], trp2[:])
                    nc.vector.memset(h2T[H : H + 1, :], 1.0)
                    ops = gpool.tile([P, C], dt.float32, tag="g", name="ops")
                    nc.tensor.matmul(
                        ops[:], lhsT=h2T[:], rhs=wfc_s[:], start=True, stop=True
                    )
                    osb = npool.tile([P, C], dt.float32, tag="osb")
                    nc.vector.tensor_copy(osb[:], ops[:])
                    nc.sync.dma_start(o_out[:], osb[:])

                node_update(t, seg, npool, gpool, nfT2, wn1_s, b1_out)

            edge_phase(1, cpts1, offs1, i_eft1, i_src1, i_oh1, we1_s,
                       h1f, i_nid1, b1_tile)

    nc.compile()
    return nc


_CACHE = {}


def kernel(**inputs):
    node_features = np.asarray(inputs["node_features"], dtype=np.float32)
    ef0 = np.asarray(inputs["edge_feat0"], dtype=np.float32)
    ef1 = np.asarray(inputs["edge_feat1"], dtype=np.float32)
    We0 = np.asarray(inputs["We0"], dtype=np.float32)
    be0 = np.asarray(inputs["be0"], dtype=np.float32)
    We1 = np.asarray(inputs["We1"], dtype=np.float32)
    be1 = np.asarray(inputs["be1"], dtype=np.float32)
    Wn0 = np.asarray(inputs["Wn0"], dtype=np.float32)
    bn0 = np.asarray(inputs["bn0"], dtype=np.float32)
    Wn1 = np.asarray(inputs["Wn1"], dtype=np.float32)
    bn1 = np.asarray(inputs["bn1"], dtype=np.float32)
    Wfc = np.asarray(inputs["Wfc"], dtype=np.float32)
    bfc = np.asarray(inputs["bfc"], dtype=np.float32)
    src0 = np.asarray(inputs["src0"]).astype(np.int64)
    dst0 = np.asarray(inputs["dst0"]).astype(np.int64)
    src1 = np.asarray(inputs["src1"]).astype(np.int64)
    dst1 = np.asarray(inputs["dst1"]).astype(np.int64)
    nid0 = np.asarray(inputs["nid0"]).astype(np.int64)
    nid1 = np.asarray(inputs["nid1"]).astype(np.int64)

    T0 = N1 // NCORES // P  # 10
    eftA0, srcA0, dstA0, cpts0, offs0, EP0, TC0 = _prep_edges(ef0, src0, dst0, N1 // NCORES, T0)
    eftA1, srcA1, dstA1, cpts1, offs1, EP1, TC1 = _prep_edges(ef1, src1, dst1, N2 // NCORES, 1)

    key = (EP0, TC0, EP1, TC1, tuple(cpts0), tuple(cpts1))
    if key not in _CACHE:
        _CACHE[key] = _build_program(cpts0, offs0, EP0, TC0, cpts1, offs1, EP1, TC1)
    nc = _CACHE[key]

    we0a = _augment(We0, be0)
    we1a = _augment(We1, be1)
    wn0a = _augment(Wn0, bn0)
    wn1a = _augment(Wn1, bn1)
    wfca = _augment(Wfc, bfc)
    nf16 = node_features.astype(BF16)
    ident = np.eye(P, dtype=np.float32).astype(BF16)
    ssel = np.zeros((P, 2), dtype=np.float32)
    ssel[0:H, 0] = 1.0
    ssel[H:P, 1] = 1.0
    ssel = ssel.astype(BF16)

    def onehots(dstA):
        # dstA: [NCORES, P, TC] local dst (PAD_SENTINEL for padding)
        # -> [NCORES, P, TC*P] bf16 with oh[c, p, ch*P + v] = (dstA==v)
        oh = (dstA[..., None] == np.arange(P, dtype=np.float32)).astype(BF16)
        return oh.reshape(NCORES, P, -1)

    ohA0 = onehots(dstA0)
    ohA1 = onehots(dstA1)

    def dup_cols(a):
        # [NCORES, P, K] -> [NCORES, P, 2K] with each column duplicated
        return np.repeat(a, 2, axis=2).copy()

    srcD0 = dup_cols(srcA0)
    srcD1 = dup_cols(srcA1)

    in_maps = []
    for c in range(NCORES):
        nid0_c = nid0[c * (N1 // NCORES) : (c + 1) * (N1 // NCORES)]
        nid1_c = nid1[c * (N2 // NCORES) : (c + 1) * (N2 // NCORES)]
        in_maps.append(
            {
                "we0a": we0a,
                "we1a": we1a,
                "wn0a": wn0a,
                "wn1a": wn1a,
                "wfca": wfca,
                "nf16": nf16,
                "eft0": eftA0[c],
                "src0i": srcD0[c],
                "oh0": ohA0[c],
                "eft1": eftA1[c],
                "src1i": srcD1[c],
                "oh1": ohA1[c],
                "nidx0": np.repeat(
                    nid0_c.reshape(T0, P).T.astype(np.int32), 2, axis=1
                ).copy(),
                "nidx1": np.repeat(
                    nid1_c.reshape(1, P).T.astype(np.int32), 2, axis=1
                ).copy(),
                "ident": ident,
                "ssel": ssel,
            }
        )

    global last_results, _LAST_IN_MAPS
    _LAST_IN_MAPS = in_maps
    res = bass_utils.run_bass_kernel_spmd(nc, in_maps, core_ids=list(range(NCORES)))
    last_results = res
    out = np.concatenate([res.results[c]["out"] for c in range(NCORES)], axis=0)
    return out.astype(np.float32)


last_results = None


def bench(inputs, iters=8):
    """Time the compiled SPMD executable with device-resident inputs.

    Returns (best_seconds, list_of_seconds). Mirrors
    bass2jax.run_bass_via_pjrt's sharded-jit construction so the jitted fn
    is built once and timed with inputs already on device.
    """
    import time
    import jax
    from jax.sharding import Mesh, PartitionSpec, NamedSharding
    from jax.experimental.shard_map import shard_map
    from concourse import bass2jax, mybir as _mb

    # run once through kernel() to populate _CACHE and build in_maps
    kernel(**inputs)
    nc = next(iter(_CACHE.values()))
    in_maps = _LAST_IN_MAPS

    bass2jax.install_neuronx_cc_hook()
    partition_name = (
        nc.partition_id_tensor.name if nc.partition_id_tensor else None
    )
    in_names, out_names, out_avals, zero_outs = [], [], [], []
    for alloc in nc.m.functions[0].allocations:
        if not isinstance(alloc, _mb.MemoryLocationSet):
            continue
        name = alloc.memorylocations[0].name
        if alloc.kind == "ExternalInput":
            if name != partition_name:
                in_names.append(name)
        elif alloc.kind == "ExternalOutput":
            shape = tuple(alloc.tensor_shape)
            dtype = _mb.dt.np(alloc.dtype)
            out_avals.append(jax.core.ShapedArray(shape, dtype))
            out_names.append(name)
            zero_outs.append(np.zeros(shape, dtype))
    n_params = len(in_names)
    n_outs = len(out_avals)
    all_in_names = list(in_names) + list(out_names)
    if partition_name is not None:
        all_in_names.append(partition_name)
    donate = tuple(range(n_params, n_params + n_outs))

    def _body(*args):
        operands = list(args)
        if partition_name is not None:
            operands.append(bass2jax.partition_id_tensor())
        outs = bass2jax._bass_exec_p.bind(
            *operands,
            out_avals=tuple(out_avals),
            in_names=tuple(all_in_names),
            out_names=tuple(out_names),
            lowering_input_output_aliases=(),
            sim_require_finite=True,
            sim_require_nnan=True,
            nc=nc,
        )
        return tuple(outs)

    devices = jax.devices()[:NCORES]
    mesh = Mesh(np.asarray(devices), ("core",))
    in_specs = (PartitionSpec("core"),) * (n_params + n_outs)
    out_specs = (PartitionSpec("core"),) * n_outs
    sharded = jax.jit(
        shard_map(
            _body, mesh=mesh, in_specs=in_specs, out_specs=out_specs,
            check_rep=False,
        ),
        donate_argnums=donate,
        keep_unused=True,
    )
    shd = NamedSharding(mesh, PartitionSpec("core"))
    concat_in = [
        jax.device_put(
            np.concatenate([np.asarray(in_maps[c][n]) for c in range(NCORES)], axis=0),
            shd,
        )
        for n in in_names
    ]
    def zeros_dev():
        return [
            jax.device_put(
                np.zeros((NCORES * z.shape[0], *z.shape[1:]), z.dtype), shd
            )
            for z in zero_outs
        ]

    # warmup (compiles)
    o = sharded(*concat_in, *zeros_dev())
    jax.block_until_ready(o)
    times = []
    for _ in range(iters):
        zs = zeros_dev()
        jax.block_until_ready(zs)
        t0 = time.perf_counter()
        o = sharded(*concat_in, *zs)
        jax.block_until_ready(o)
        times.append(time.perf_counter() - t0)
    return min(times), times


if __name__ == "__main__":
    import reference

    inp = {k: np.asarray(v) for k, v in reference.setup_inputs().items()}
    expected = np.asarray(reference.reference(**reference.setup_inputs()))
    actual = kernel(**inp)
    err = np.abs(actual - expected).max() / (np.abs(expected).max() + 1e-9)
    print("Relative error:", err)
